# revision 9
# baseline (speedup 1.0000x reference)
"""Trainium2 Bass kernel for Chebyshev (L-inf) "convolution".

Math (see reference):
  out[b,co,h,w] = max_n |weights[co,n] - x_pad[b, c(co,n), h+di(co,n), w+dj(co,n)]| + bias[co]
  where conn_idx[co,n] = c*9 + di*3 + dj and x_pad is replicate-padded by 1.

Strategy (8 NeuronCores, batch-sharded: 4 images per core, processed as 2
pairs of images):
  1. Per image: load x into SBUF, cast+replicate-pad into a bf16 pair tile
     [C=64, 2*66*66]; one contiguous DMA stores the pair to DRAM scratch.
  2. Per (pair, tap): ONE indirect DMA with a [128, 2] offset AP gathers the
     shifted 4222-element span for both images of the pair (256 descriptors).
     Offset for (partition co, image i): i*64*4356 + c*4356 + di*66 + dj.
  3. Compute per pair (all ops sized [128, 2*4096] bf16):
       t0 = Abs(G0 - w0)        ScalarE activation (bias = -w0)
       t1 = Abs(G1 - w1)        ScalarE
       m01 = max(t0, t1)        Pool (GpSimd) tensor_tensor
       t2 = Abs(G2 - w2)        ScalarE
       d3 = G3 + (-w3)          VectorE tensor_scalar
       t3 = max(-d3, d3)        VectorE scalar_tensor_tensor (mult -1, max)
       m23 = max(t2, t3)        VectorE
       mf  = max(m01, m23)      VectorE  -> stored bf16
  4. bias add + f32 upcast happen on the host (exact f32 add, fused with the
     unshard pass).  Output DMA is bf16: halves the store traffic.

The kernel is DMA-byte-bound (~28 MB/core across 16 DMA engines @ ~23 GB/s
= ~76 us floor); engine work (~40 us each) hides underneath.
"""

import numpy as np

B, CIN, H, W = 32, 64, 64, 64
COUT, NCONN = 128, 4
KH, KW = 3, 3
NCORES = 8
BL = B // NCORES            # 4 images per core
NPAIR = BL // 2             # 2 pairs per core
PH, PW = H + 2, W + 2       # 66 x 66 replicate-padded planes
PLANE = PH * PW             # 4356
S = H * W                   # 4096
SPAN = (H - 1) * PW + W     # 4222: span holding one shifted 64x64 window
GPAD = SPAN + 2             # 4224 == 64*66 (even) per-image gather width

_CACHE = {}


def _build_program():
    import concourse.bass as bass
    import concourse.bacc as bacc
    import concourse.mybir as mybir
    from concourse.tile import TileContext, add_dep_helper

    f32 = mybir.dt.float32
    bf16 = mybir.dt.bfloat16
    i32 = mybir.dt.int32
    Alu = mybir.AluOpType
    Act = mybir.ActivationFunctionType

    nc = bacc.Bacc("TRN2", target_bir_lowering=False, debug=False)

    x_ext = nc.dram_tensor("x", (BL, CIN, H, W), f32, kind="ExternalInput").ap()
    wneg_ext = nc.dram_tensor("wneg", (COUT, NCONN), f32, kind="ExternalInput").ap()
    # per (pair, tap): a [128, 2] int32 offset pair at cols [(P*NCONN+n)*8, +2)
    gidx_ext = nc.dram_tensor(
        "gidx", (COUT, NPAIR * NCONN * 8), i32, kind="ExternalInput"
    ).ap()
    out_ext = [
        nc.dram_tensor(f"out{p}", (2, COUT, H, W), bf16, kind="ExternalOutput").ap()
        for p in range(NPAIR)
    ]
    xpad_all = nc.dram_tensor("xpad", (BL * CIN * PLANE, 1), bf16)
    # per-pair view [pair][c, i, plane-pos]
    xpad_pair = xpad_all.ap().rearrange(
        "(P i c p) one -> P c i (p one)", P=NPAIR, i=2, c=CIN, p=PLANE
    )

    with TileContext(nc, pool_alloc_mode="queue") as tc:
        with (
            tc.tile_pool(name="const", bufs=1) as cpool,
            tc.tile_pool(name="xs", bufs=2) as xspool,
            tc.tile_pool(name="xp", bufs=2) as xppool,
            tc.tile_pool(name="g", bufs=2) as gpool,
            tc.tile_pool(name="t", bufs=3) as tpool,
            tc.tile_pool(name="m", bufs=3) as mpool,
        ):
            wneg_sb = cpool.tile([COUT, NCONN], f32)
            nc.sync.dma_start(out=wneg_sb[:], in_=wneg_ext)
            gidx_sb = cpool.tile([COUT, NPAIR * NCONN * 8], i32)
            nc.sync.dma_start(out=gidx_sb[:], in_=gidx_ext)

            for P in range(NPAIR):
                # --- padded bf16 planes for the two images of pair P ---
                XP = xppool.tile([CIN, 2 * PLANE], bf16, tag="xp")
                XPv = XP[:].rearrange("c (i h w) -> c i h w", i=2, h=PH, w=PW)
                for i in range(2):
                    XSB = xspool.tile([CIN, S], f32, tag="xsb")
                    nc.sync.dma_start(
                        out=XSB[:],
                        in_=x_ext[2 * P + i].rearrange("c h w -> c (h w)"),
                    )
                    nc.vector.tensor_copy(
                        out=XPv[:, i, 1 : H + 1, 1 : W + 1],
                        in_=XSB[:].rearrange("c (h w) -> c h w", h=H, w=W),
                    )
                    nc.vector.tensor_copy(
                        out=XPv[:, i, 1 : H + 1, 0:1],
                        in_=XPv[:, i, 1 : H + 1, 1:2],
                    )
                    nc.vector.tensor_copy(
                        out=XPv[:, i, 1 : H + 1, PW - 1 : PW],
                        in_=XPv[:, i, 1 : H + 1, PW - 2 : PW - 1],
                    )
                    nc.vector.tensor_copy(
                        out=XPv[:, i, 0:1, :], in_=XPv[:, i, 1:2, :]
                    )
                    nc.vector.tensor_copy(
                        out=XPv[:, i, PH - 1 : PH, :],
                        in_=XPv[:, i, PH - 2 : PH - 1, :],
                    )
                store = nc.sync.dma_start(
                    out=xpad_pair[P],
                    in_=XP[:].rearrange("c (i p) -> c i p", i=2, p=PLANE),
                )

                # --- per tap: one 2-image indirect span gather (256 descs) ---
                gvs = []
                for n in range(NCONN):
                    k = (P * NCONN + n) * 8
                    gt = gpool.tile([COUT, 2 * GPAD], bf16, tag="g")
                    gtv = gt[:].rearrange("p (i g) -> p i g", i=2)
                    for i in range(2):
                        gather = nc.gpsimd.indirect_dma_start(
                            out=gtv[:, i, 0:SPAN],
                            out_offset=None,
                            in_=xpad_all.ap(),
                            in_offset=bass.IndirectOffsetOnAxis(
                                ap=gidx_sb[:, k + i : k + i + 1], axis=0
                            ),
                        )
                        add_dep_helper(
                            gather.ins, store.ins, reason="gather reads xpad pair"
                        )
                    gvs.append(
                        gt[:].rearrange(
                            "p (i h w) -> p i h w", i=2, h=H, w=PW
                        )[:, :, :, 0:W]
                    )

                def tview(tile):
                    return tile[:].rearrange(
                        "p (i h w) -> p i h w", i=2, h=H, w=W
                    )

                # t0/t1/t2 = |G - w| on ScalarE
                t0 = tpool.tile([COUT, 2 * S], bf16, tag="t")
                nc.scalar.activation(
                    out=tview(t0), in_=gvs[0], func=Act.Abs,
                    bias=wneg_sb[:, 0:1], scale=1.0,
                )
                t1 = tpool.tile([COUT, 2 * S], bf16, tag="t")
                nc.scalar.activation(
                    out=tview(t1), in_=gvs[1], func=Act.Abs,
                    bias=wneg_sb[:, 1:2], scale=1.0,
                )
                m01 = mpool.tile([COUT, 2 * S], bf16, tag="m")
                nc.vector.tensor_tensor(
                    out=m01[:], in0=t0[:], in1=t1[:], op=Alu.max
                )
                t2 = tpool.tile([COUT, 2 * S], bf16, tag="t")
                nc.scalar.activation(
                    out=tview(t2), in_=gvs[2], func=Act.Abs,
                    bias=wneg_sb[:, 2:3], scale=1.0,
                )
                # tap 3 on VectorE: d3 = G3 + (-w3); t3 = max(-d3, d3)
                d3 = tpool.tile([COUT, 2 * S], bf16, tag="t")
                nc.vector.tensor_scalar(
                    out=tview(d3), in0=gvs[3],
                    scalar1=wneg_sb[:, 3:4], scalar2=None, op0=Alu.add,
                )
                t3 = tpool.tile([COUT, 2 * S], bf16, tag="t")
                nc.vector.scalar_tensor_tensor(
                    out=t3[:], in0=d3[:], scalar=-1.0, in1=d3[:],
                    op0=Alu.mult, op1=Alu.max,
                )
                m23 = mpool.tile([COUT, 2 * S], bf16, tag="m")
                nc.vector.tensor_tensor(
                    out=m23[:], in0=t2[:], in1=t3[:], op=Alu.max
                )
                mf = mpool.tile([COUT, 2 * S], bf16, tag="m")
                nc.vector.tensor_tensor(
                    out=mf[:], in0=m01[:], in1=m23[:], op=Alu.max
                )
                nc.sync.dma_start(
                    out=out_ext[P].rearrange("i c h w -> c i (h w)"),
                    in_=mf[:].rearrange("p (i s) -> p i s", i=2),
                )
    nc.compile()
    return nc


def _host_inputs(x, weights, bias, conn_idx):
    """Per-core input maps (host-side prep: shard x, derive -w and pairwise
    gather offsets from the tiny weight/index tensors)."""
    ci = np.asarray(conn_idx).astype(np.int64)          # [COUT, NCONN]
    c = ci // (KH * KW)
    rem = ci % (KH * KW)
    di = rem // KW
    dj = rem % KW
    # element offset into one padded image [64, 66, 66]: c*4356 + di*66 + dj
    offs = (c * PLANE + di * PW + dj).astype(np.int64)          # [COUT, NCONN]
    gidx = np.zeros((COUT, NPAIR * NCONN * 8), dtype=np.int32)
    for P in range(NPAIR):
        for n in range(NCONN):
            k = (P * NCONN + n) * 8
            for i in range(2):
                gidx[:, k + i] = (
                    (2 * P + i) * CIN * PLANE + offs[:, n]
                ).astype(np.int32)
    wneg = (-np.asarray(weights)).astype(np.float32)
    x = np.ascontiguousarray(np.asarray(x), dtype=np.float32)
    in_maps = []
    for kcore in range(NCORES):
        in_maps.append(
            {
                "x": x[kcore * BL : (kcore + 1) * BL],
                "wneg": wneg,
                "gidx": gidx,
            }
        )
    return in_maps


def kernel(x, weights, bias, conn_idx):
    from concourse.bass_utils import run_bass_kernel_spmd

    if "nc" not in _CACHE:
        _CACHE["nc"] = _build_program()
    nc = _CACHE["nc"]
    in_maps = _host_inputs(x, weights, bias, conn_idx)
    res = run_bass_kernel_spmd(nc, in_maps, list(range(NCORES)))
    bias_f = np.asarray(bias, dtype=np.float32).reshape(1, COUT, 1, 1)
    outs = []
    for k in range(NCORES):
        for P in range(NPAIR):
            a = np.asarray(res.results[k][f"out{P}"]).astype(np.float32)
            outs.append(a + bias_f)    # exact f32 bias add on host
    return np.concatenate(outs, axis=0)


if __name__ == "__main__":
    nc = _build_program()
    print("program built OK")
